# revision 41
# baseline (speedup 1.0000x reference)
"""Chamfer (MeshLoss) kernel for 8 Trainium2 NeuronCores.

Problem: vertices [4,3,64,32,64], pc [4,3,8192] ->
  top surface v = (vertices[:,:,:,-1,:] - 0.5)*2 reshaped to [B, N=4096, 3]
  p = pc^T [B, M=8192, 3], mask = point not all-zero
  d[i,j] = |v_i|^2 + |p_j|^2 - 2 v.p
  loss_b = mean_i min_valid_j d  +  sum_valid_j (min_i d) / n_valid
  out = mean_b loss_b   (scalar f32)

Sharding: core c -> (sample b = c//2, pc-half h = c%2).  Each core computes
the full [N x M/2] block of scaled negated distances in PSUM via a K=5 fp32
matmul that emits
  -d[i,j]/4 = T.P - (psq + 2*sp)_j/4 - BIG*invalid_j - vsq_i/4
(T, P raw input coords; the affine (x-0.5)*2 and all norms are folded into
the extra contraction rows, built on-device; /4 folded so the coordinate
rows stay raw).  Per [128,2048] PSUM group:
  - ACT stages PSUM -> SBUF fp16 (absorbs the mandatory psum read)
  - DVE row-chain: tensor_scalar(max, -big) @4x with accum_out -> row-max
  - DVE col-chain: tensor_tensor max @2x into a running fp16 buffer
Tail: PE-transpose of the colmax buffer + DVE reduce -> per-j max.
Host combines the per-core [128,32] outputs (max across core pairs, *-4,
masking, means).
"""

import numpy as np

import concourse.bass as bass
import concourse.mybir as mybir
import concourse.tile as tile
from concourse.bass_utils import run_bass_kernel_spmd

F32 = mybir.dt.float32
F16 = mybir.dt.float16
ALU = mybir.AluOpType
AF = mybir.ActivationFunctionType

B = 4
N = 4096      # mesh-top points per sample
M = 8192      # cloud points per sample
MH = M // 2   # per-core pc half
N_CORES = 8
BIG = 8000.0          # mask penalty in -d/4 units: below any valid value
MM_DT = mybir.dt.float32r   # matmul operand view: f32r = full-rate on PE
NEG_INIT = -60000.0   # fp16-representable "-inf" init for max chains
SCALE = 2.0
OFFSET = 0.5


def build_nc(n=N, mh=MH):
    """Build the single-core Bass program (SPMD: same program, per-core data).

    n  : number of v points handled by this core (full N)
    mh : number of p points handled by this core (half of M)
    """
    assert n % 128 == 0 and mh % 512 == 0
    nt = n // 128            # i-tiles
    jt = mh // 128           # j-tiles (for dist2 output)
    G = min(2048, mh)        # psum group columns (4 banks)
    ng = mh // G             # groups per i-tile
    gc = G // 512            # matmuls per group

    nc = bass.Bass("TRN2", target_bir_lowering=False, debug=False,
                   num_devices=N_CORES)

    # l_base: rows 0-2 raw T, row 3 ones, row 4 placeholder (-> -vsq/4)
    # r_base: rows 0-2 raw P, row 3 placeholder (-> -(psq+2sp)/4 - BIGmask),
    #         row 4 ones
    l_base = nc.dram_tensor("l_base", [5, n], MM_DT, kind="ExternalInput").ap()
    r_base = nc.dram_tensor("r_base", [5, mh], MM_DT,
                            kind="ExternalInput").ap()
    t_good = nc.dram_tensor("t_good", [n // 128, 384], F32,
                            kind="ExternalInput").ap()
    p_good = nc.dram_tensor("p_good", [mh // 128, 384], F32,
                            kind="ExternalInput").ap()
    ident = nc.dram_tensor("ident", [128, 128], F16, kind="ExternalInput").ap()
    outneg = nc.dram_tensor("outneg", [128, nt + jt], F32,
                            kind="ExternalOutput").ap()

    with tile.TileContext(nc) as tc:
        with tc.tile_pool(name="const", bufs=1) as cpool, \
             tc.tile_pool(name="stage", bufs=6) as spool, \
             tc.tile_pool(name="ps", bufs=2, space="PSUM") as pspool:

            # ---- persistent SBUF tensors ----
            L = cpool.tile([5, n], MM_DT, tag="L")        # lhsT (i side)
            R = cpool.tile([5, mh], MM_DT, tag="R")       # rhs  (j side)
            idt = cpool.tile([128, 128], F16, tag="idt")
            cmax = cpool.tile([128, mh], F16, tag="cmax")
            obuf = cpool.tile([128, nt + jt], F32, tag="obuf")
            d1buf = obuf[:, 0:nt]
            d2buf = obuf[:, nt:nt + jt]
            rowp = cpool.tile([128, nt * ng], F32, tag="rowp")

            nc.sync.dma_start(idt[:], ident)
            # keep DMA targets disjoint from the derived-row DMAs below:
            # PSEUDO_DMA_DIRECT2D embeds at most ONE sem wait, so each of
            # these must have <= 1 dependency.
            nc.sync.dma_start(L[0:4, :], l_base[0:4, :])
            nc.sync.dma_start(R[0:3, :], r_base[0:3, :])
            nc.sync.dma_start(R[4:5, :], r_base[4:5, :])

            # ---- prep: derived rows ----
            tg = cpool.tile([n // 128, 384], F32, tag="tg")
            pg = cpool.tile([mh // 128, 384], F32, tag="pg")
            nc.sync.dma_start(tg[:], t_good)
            nc.sync.dma_start(pg[:], p_good)

            # i side row 4 = -vsq/4 where v = 2t - 1
            v2 = cpool.tile([n // 128, 384], F32, tag="v2")
            nc.vector.tensor_scalar(v2[:], tg[:], SCALE, -SCALE * OFFSET,
                                    op0=ALU.mult, op1=ALU.add)
            vsq = cpool.tile([n // 128, 384], F32, tag="vsq")
            nc.vector.tensor_tensor(vsq[:], v2[:], v2[:], op=ALU.mult)
            va = cpool.tile([n // 128, 128], F32, tag="va")
            nc.vector.tensor_tensor(va[:], vsq[:, 0:128], vsq[:, 128:256],
                                    op=ALU.add)
            vb = cpool.tile([n // 128, 128], F32, tag="vb")
            nc.vector.tensor_tensor(vb[:], va[:], vsq[:, 256:384], op=ALU.add)
            nvs = cpool.tile([n // 128, 128], MM_DT, tag="nvs")
            nc.vector.tensor_scalar_mul(nvs[:], vb[:], -0.25)
            nc.sync.dma_start(L[4:5, :], nvs[:])

            # j side row 3 = -(psq + 2sp)/4 - BIG*invalid
            psq_e = cpool.tile([mh // 128, 384], F32, tag="psq_e")
            nc.vector.tensor_tensor(psq_e[:], pg[:], pg[:], op=ALU.mult)
            pa = cpool.tile([mh // 128, 128], F32, tag="pa")
            nc.vector.tensor_tensor(pa[:], psq_e[:, 0:128], psq_e[:, 128:256],
                                    op=ALU.add)
            psq = cpool.tile([mh // 128, 128], F32, tag="psq")
            nc.vector.tensor_tensor(psq[:], pa[:], psq_e[:, 256:384],
                                    op=ALU.add)
            sa = cpool.tile([mh // 128, 128], F32, tag="sa")
            nc.vector.tensor_tensor(sa[:], pg[:, 0:128], pg[:, 128:256],
                                    op=ALU.add)
            sp = cpool.tile([mh // 128, 128], F32, tag="sp")
            nc.vector.tensor_tensor(sp[:], sa[:], pg[:, 256:384], op=ALU.add)
            psq4 = cpool.tile([mh // 128, 128], F32, tag="psq4")
            nc.vector.tensor_scalar_mul(psq4[:], psq[:], 0.25)
            # nb = (sp * -0.5) - psq/4  =  -(psq + 2*sp)/4
            nb = cpool.tile([mh // 128, 128], F32, tag="nb")
            nc.vector.scalar_tensor_tensor(nb[:], sp[:], -0.5, psq4[:],
                                           op0=ALU.mult, op1=ALU.subtract)
            # invalid point <=> psq == 0 ; r3 = (is0 * -BIG) + nb
            is0 = cpool.tile([mh // 128, 128], F32, tag="is0")
            nc.vector.tensor_scalar(is0[:], psq[:], 0.0, None,
                                    op0=ALU.is_equal)
            r3 = cpool.tile([mh // 128, 128], MM_DT, tag="r3")
            nc.vector.scalar_tensor_tensor(r3[:], is0[:], -BIG, nb[:],
                                           op0=ALU.mult, op1=ALU.add)
            nc.sync.dma_start(R[3:4, :], r3[:])

            # ---- init col-max accumulator ----
            nc.gpsimd.memset(cmax[:], NEG_INIT)

            # absorb the cmax-memset (Pool) semaphore into the DVE clock
            # once, so col-chain TTs carry only their other wait
            pscr = cpool.tile([1, 8], F16, tag="pscr")
            gscr = cpool.tile([1, 8], F32, tag="gscr")
            nc.vector.tensor_copy(pscr[0:1, 0:1], cmax[0:1, 0:1])

            # ---- wait-spreaders: absorb one DMA-queue semaphore each so
            # real matmuls carry <=1 embedded wait (S3_LW struct limit) ----
            wp = pspool.tile([128, G], F32, tag="pt")
            for ap_ in (L[0:4, 0:1], L[0:5, 0:1], R[0:3, 0:1],
                        R[0:4, 0:1], R[0:5, 0:1]):
                nc.tensor.matmul(wp[0:1, 0:1], ap_.bitcast(F32),
                                 ap_.bitcast(F32), start=True, stop=True)
            nc.tensor.matmul(wp[0:1, 0:1], idt[0:1, 0:1], idt[0:1, 0:1],
                             start=True, stop=True)

            # ---- main loop ----
            for it in range(nt):
                sts = []
                for g in range(ng):
                    pt = pspool.tile([128, G], F32, tag="pt")
                    for c in range(gc):
                        j0 = g * G + c * 512
                        nc.tensor.matmul(
                            pt[:, c * 512:(c + 1) * 512],
                            L[0:5, it * 128:(it + 1) * 128],
                            R[0:5, j0:j0 + 512],
                            start=True, stop=True)
                    # ACT: stage psum -> sbuf fp16.  The pre-touch write
                    # (to the disjoint pad column) absorbs the stage-slot
                    # WAR (DVE readers of the tile a few groups ago) so the
                    # real stage carries only the PE wait -- walrus embeds
                    # at most one sem wait per instruction.
                    st = spool.tile([128, G + 8], F16, tag="st")
                    nc.scalar.activation(st[0:1, G:G + 1], idt[0:1, 0:1],
                                         AF.Copy)
                    nc.scalar.activation(st[:, 0:G], pt[:], AF.Copy)
                    # col chain: cmax = max(cmax, staged)  (before the
                    # in-place row folding below mangles st)
                    nc.vector.tensor_tensor(cmax[:, g * G:(g + 1) * G],
                                            cmax[:, g * G:(g + 1) * G],
                                            st[:, 0:G], op=ALU.max)
                    sts.append(st)
                # DVE row chain: pairwise-max folds at the 16-bit 2x TT
                # rate (first fold across the ng staged tiles), then one
                # narrow 1x reduce.  (Full-width reduce is 1x-capped;
                # fused tensor_scalar+accum too; TTR doesn't compile.)
                st = sts[0]
                for o in sts[1:]:
                    nc.vector.tensor_tensor(st[:, 0:G], st[:, 0:G],
                                            o[:, 0:G], op=ALU.max)
                h1, h2, h3 = G // 2, G // 4, G // 8
                nc.vector.tensor_tensor(st[:, 0:h1], st[:, 0:h1],
                                        st[:, h1:G], op=ALU.max)
                nc.vector.tensor_tensor(st[:, 0:h2], st[:, 0:h2],
                                        st[:, h2:h1], op=ALU.max)
                nc.vector.tensor_tensor(st[:, 0:h3], st[:, 0:h3],
                                        st[:, h3:h2], op=ALU.max)
                nc.vector.tensor_reduce(
                    d1buf[:, it:it + 1], st[:, 0:h3],
                    axis=mybir.AxisListType.X, op=ALU.max)

            # ---- tail: per-j max of cmax via PE transpose + DVE reduce.
            # 4 transposes share one PSUM tile; one 3D-AP reduce covers all
            # 4 (amortizes the 120-cycle PSUM-read init per DVE op).
            tb = 4
            for t0 in range(0, jt, tb):
                k = min(tb, jt - t0)
                ptt = pspool.tile([128, 128 * tb], F16, tag="pt")
                for q in range(k):
                    nc.tensor.transpose(
                        ptt[:, q * 128:(q + 1) * 128],
                        cmax[:, (t0 + q) * 128:(t0 + q + 1) * 128], idt[:])
                nc.vector.tensor_reduce(
                    d2buf[:, t0:t0 + k],
                    ptt[:, 0:k * 128].rearrange("p (a b) -> p a b", b=128),
                    axis=mybir.AxisListType.X, op=ALU.max)

            nc.gpsimd.dma_start(outneg, obuf[:])

    strip_redundant_waits(nc)
    return nc


def strip_redundant_waits(nc):
    """Transitively-implied semaphore-wait elimination.

    Tile emits per-instruction wait lists without transitive reduction
    (documented: "Tile doesn't track that syncing on engine X told us
    about Y").  walrus's fp32-matmul lowering (S3_LW) and direct2d DMA
    structs can embed only ONE wait, so a slot-reuse matmul carrying
    [ACT>=a, PE>=p] fails codegen even though the PE wait is implied by
    the ACT wait (the ACT instruction itself waited on PE>=p).

    Soundness: a wait (S>=v) may be dropped iff it is guaranteed by the
    union of (a) knowledge inherited from the previous instruction on
    the same in-order engine, and (b) completion-knowledge of the
    instructions that perform the other waits' target increments.
    Completion of an in-order engine's instruction implies completion
    (and sem updates) of all earlier instructions on that engine.  DMA
    transfers complete out of order w.r.t. the issuing engine, so each
    DMA instruction is its own "engine".
    """
    import concourse.mybir as mb

    insts = []
    for blk in nc.m.functions[0].blocks:
        insts.extend(list(blk.instructions))
    if True:
        n = len(insts)
        # engine key per instruction (DMA transfers are their own proc)
        ekeys = []
        for idx, i in enumerate(insts):
            if type(i).__name__ in ("InstDMACopy", "InstLoad", "InstSave"):
                ekeys.append(("dma", idx))
            else:
                ekeys.append(("eng", str(getattr(i, "engine", idx))))
        prev_on_eng = {}
        prev_idx = [None] * n
        for idx in range(n):
            k = ekeys[idx]
            prev_idx[idx] = prev_on_eng.get(k)
            prev_on_eng[k] = idx
        # cumulative sem updates in schedule order; sems that are ever
        # decremented or register-updated are excluded (non-monotone).
        bad_sems = set()
        for i in insts:
            si = i.sync_info
            if not si:
                continue
            for u in si.on_update:
                if u.update_mode not in ("sem-add-imm", "sem-inc")                         or u.update_reg is not None:
                    bad_sems.add(u.ant_name)
        upd_timeline = {}
        cums = {}
        upd_of = [None] * n  # idx -> list[(sem, cum_after)]
        for idx, i in enumerate(insts):
            si = i.sync_info
            if not si:
                upd_of[idx] = []
                continue
            ups = []
            for u in si.on_update:
                if u.ant_name in bad_sems:
                    continue
                amt = 1 if u.update_mode == "sem-inc" else u.update_value
                c = cums.get(u.ant_name, 0) + amt
                cums[u.ant_name] = c
                upd_timeline.setdefault(u.ant_name, []).append((c, idx))
                ups.append((u.ant_name, c))
            upd_of[idx] = ups

        def inc_idx(sem, v):
            tl = upd_timeline.get(sem)
            if not tl:
                return None
            for c, idx in tl:
                if c >= v:
                    return idx
            return None

        D_cache = {}
        C_cache = {}

        def merge(dst, src):
            for s, v in src.items():
                if dst.get(s, -1) < v:
                    dst[s] = v

        def D(idx):
            if idx in D_cache:
                return D_cache[idx]
            D_cache[idx] = {}   # cycle guard
            out = {}
            p = prev_idx[idx]
            if p is not None:
                merge(out, D(p))
            si = insts[idx].sync_info
            if si:
                for w in si.on_wait:
                    if w.wait_mode != "sem-ge-imm" or w.wait_reg is not None                             or w.ant_name in bad_sems:
                        continue
                    j = inc_idx(w.ant_name, w.wait_value)
                    if j is not None and j < idx:
                        merge(out, C(j))
                    if out.get(w.ant_name, -1) < w.wait_value:
                        out[w.ant_name] = w.wait_value
            D_cache[idx] = out
            return out

        def C(idx):
            if idx in C_cache:
                return C_cache[idx]
            C_cache[idx] = {}   # cycle guard
            out = dict(D(idx))
            # completion of idx implies completion of all earlier same-eng
            k = ekeys[idx]
            j = idx
            while j is not None:
                for s, c in upd_of[j]:
                    if out.get(s, -1) < c:
                        out[s] = c
                j = prev_idx[j]
            C_cache[idx] = out
            return out

        for idx, i in enumerate(insts):
            si = i.sync_info
            if not si or len(si.on_wait) <= 1:
                continue
            waits = list(si.on_wait)
            if any(w.wait_mode != "sem-ge-imm" or w.wait_reg is not None
                   for w in waits):
                continue
            keep = []
            for wi, w in enumerate(waits):
                if w.ant_name in bad_sems:
                    keep.append(w)
                    continue
                know = {}
                p = prev_idx[idx]
                if p is not None:
                    merge(know, D(p))
                for wj, w2 in enumerate(waits):
                    if wj == wi or w2.ant_name in bad_sems:
                        continue
                    j = inc_idx(w2.ant_name, w2.wait_value)
                    if j is not None and j < idx:
                        merge(know, C(j))
                    if know.get(w2.ant_name, -1) < w2.wait_value:
                        know[w2.ant_name] = w2.wait_value
                if know.get(w.ant_name, -1) >= w.wait_value:
                    continue    # implied -> drop
                keep.append(w)
            if len(keep) < len(waits):
                i.sync_info = mb.SyncInfo(on_wait=keep,
                                          on_update=list(si.on_update))


_NC_CACHE = {}


def _get_nc(n=N, mh=MH):
    key = (n, mh)
    if key not in _NC_CACHE:
        _NC_CACHE[key] = build_nc(n, mh)
    return _NC_CACHE[key]


def make_in_maps(vertices, pc, n=N, mh=MH):
    vertices = np.asarray(vertices)
    pc = np.asarray(pc)
    b_total = vertices.shape[0]
    top = vertices[:, :, :, -1, :].reshape(b_total, 3, -1)[:, :, :n]
    top = np.ascontiguousarray(top, dtype=np.float32)
    ident = np.eye(128, dtype=np.float16)
    in_maps = []
    for c in range(N_CORES):
        b, h = divmod(c, 2)
        b = b % b_total
        t_raw = top[b]
        p_raw = np.ascontiguousarray(pc[b][:, h * mh:(h + 1) * mh],
                                     dtype=np.float32)
        l_base = np.zeros((5, n), np.float32)
        l_base[0:3] = t_raw
        l_base[3] = 1.0
        r_base = np.zeros((5, mh), np.float32)
        r_base[0:3] = p_raw
        r_base[4] = 1.0
        t_g = np.ascontiguousarray(
            t_raw.reshape(3, n // 128, 128).transpose(1, 0, 2)
            .reshape(n // 128, 384))
        p_g = np.ascontiguousarray(
            p_raw.reshape(3, mh // 128, 128).transpose(1, 0, 2)
            .reshape(mh // 128, 384))
        in_maps.append({
            "l_base": l_base, "r_base": r_base,
            "t_good": t_g, "p_good": p_g,
            "ident": ident,
        })
    return in_maps


def combine(results, pc, n=N, mh=MH):
    """Combine per-core [128, nt]/[128, jt] outputs (of -d/4) into the loss."""
    pc = np.asarray(pc)
    losses = []
    for b in range(pc.shape[0]):
        nt, jt = n // 128, mh // 128
        r0, r1 = results[2 * b], results[2 * b + 1]
        rneg = np.maximum(r0["outneg"][:, 0:nt], r1["outneg"][:, 0:nt])
        dist1 = (-4.0 * rneg.T.reshape(n)).astype(np.float64)
        dist2 = np.concatenate([
            (-4.0 * r["outneg"][:, nt:nt + jt].T.reshape(mh))
            .astype(np.float64) for r in (r0, r1)])
        mask = ~np.all(pc[b] == 0.0, axis=0)
        n_valid = max(int(mask.sum()), 1)
        losses.append(dist1.mean() + dist2[mask].sum() / n_valid)
    return np.asarray(np.mean(losses), dtype=np.float32)


def kernel(vertices, pc):
    nc = _get_nc()
    in_maps = make_in_maps(vertices, pc)
    res = run_bass_kernel_spmd(nc, in_maps, list(range(N_CORES))).results
    return combine(res, pc)


# revision 43
# speedup vs baseline: 1.0019x; 1.0019x over previous
"""Chamfer (MeshLoss) kernel for 8 Trainium2 NeuronCores.

Problem: vertices [4,3,64,32,64], pc [4,3,8192] ->
  top surface v = (vertices[:,:,:,-1,:] - 0.5)*2 reshaped to [B, N=4096, 3]
  p = pc^T [B, M=8192, 3], mask = point not all-zero
  d[i,j] = |v_i|^2 + |p_j|^2 - 2 v.p
  loss_b = mean_i min_valid_j d  +  sum_valid_j (min_i d) / n_valid
  out = mean_b loss_b   (scalar f32)

Sharding: core c -> (sample b = c//2, pc-half h = c%2).  Each core computes
the full [N x M/2] block of scaled negated distances in PSUM via a K=5 fp32
matmul that emits
  -d[i,j]/4 = T.P - (psq + 2*sp)_j/4 - BIG*invalid_j - vsq_i/4
(T, P raw input coords; the affine (x-0.5)*2 and all norms are folded into
the extra contraction rows, built on-device; /4 folded so the coordinate
rows stay raw).  Per [128,2048] PSUM group:
  - ACT stages PSUM -> SBUF fp16 (absorbs the mandatory psum read)
  - DVE row-chain: tensor_scalar(max, -big) @4x with accum_out -> row-max
  - DVE col-chain: tensor_tensor max @2x into a running fp16 buffer
Tail: PE-transpose of the colmax buffer + DVE reduce -> per-j max.
Host combines the per-core [128,32] outputs (max across core pairs, *-4,
masking, means).
"""

import numpy as np

import concourse.bass as bass
import concourse.mybir as mybir
import concourse.tile as tile
from concourse.bass_utils import run_bass_kernel_spmd

F32 = mybir.dt.float32
F16 = mybir.dt.float16
ALU = mybir.AluOpType
AF = mybir.ActivationFunctionType

B = 4
N = 4096      # mesh-top points per sample
M = 8192      # cloud points per sample
MH = M // 2   # per-core pc half
N_CORES = 8
BIG = 8000.0          # mask penalty in -d/4 units: below any valid value
MM_DT = mybir.dt.float32r   # matmul operand view: f32r = full-rate on PE
NEG_INIT = -60000.0   # fp16-representable "-inf" init for max chains
SCALE = 2.0
OFFSET = 0.5


def build_nc(n=N, mh=MH):
    """Build the single-core Bass program (SPMD: same program, per-core data).

    n  : number of v points handled by this core (full N)
    mh : number of p points handled by this core (half of M)
    """
    assert n % 128 == 0 and mh % 512 == 0
    nt = n // 128            # i-tiles
    jt = mh // 128           # j-tiles (for dist2 output)
    G = min(2048, mh)        # psum group columns (4 banks)
    ng = mh // G             # groups per i-tile
    gc = G // 512            # matmuls per group

    nc = bass.Bass("TRN2", target_bir_lowering=False, debug=False,
                   num_devices=N_CORES)

    # l_base: rows 0-2 raw T, row 3 ones, row 4 placeholder (-> -vsq/4)
    # r_base: rows 0-2 raw P, row 3 placeholder (-> -(psq+2sp)/4 - BIGmask),
    #         row 4 ones
    l_base = nc.dram_tensor("l_base", [5, n], MM_DT, kind="ExternalInput").ap()
    r_base = nc.dram_tensor("r_base", [5, mh], MM_DT,
                            kind="ExternalInput").ap()
    t_good = nc.dram_tensor("t_good", [n // 128, 384], F32,
                            kind="ExternalInput").ap()
    p_good = nc.dram_tensor("p_good", [mh // 128, 384], F32,
                            kind="ExternalInput").ap()
    ident = nc.dram_tensor("ident", [128, 128], F16, kind="ExternalInput").ap()
    outneg = nc.dram_tensor("outneg", [128, nt + jt], F32,
                            kind="ExternalOutput").ap()

    with tile.TileContext(nc) as tc:
        with tc.tile_pool(name="const", bufs=1) as cpool, \
             tc.tile_pool(name="stage", bufs=6) as spool, \
             tc.tile_pool(name="ps", bufs=2, space="PSUM") as pspool:

            # ---- persistent SBUF tensors ----
            L = cpool.tile([5, n], MM_DT, tag="L")        # lhsT (i side)
            R = cpool.tile([5, mh], MM_DT, tag="R")       # rhs  (j side)
            idt = cpool.tile([128, 128], F16, tag="idt")
            cmax = cpool.tile([128, mh], F16, tag="cmax")
            obuf = cpool.tile([128, nt + jt], F32, tag="obuf")
            d1buf = obuf[:, 0:nt]
            d2buf = obuf[:, nt:nt + jt]
            rowp = cpool.tile([128, nt * ng], F32, tag="rowp")

            nc.gpsimd.dma_start(idt[:], ident)
            # keep DMA targets disjoint from the derived-row DMAs below:
            # PSEUDO_DMA_DIRECT2D embeds at most ONE sem wait, so each of
            # these must have <= 1 dependency.
            nc.sync.dma_start(L[0:4, :], l_base[0:4, :])
            nc.scalar.dma_start(R[0:3, :], r_base[0:3, :])
            nc.gpsimd.dma_start(R[4:5, :], r_base[4:5, :])

            # ---- prep: derived rows ----
            tg = cpool.tile([n // 128, 384], F32, tag="tg")
            pg = cpool.tile([mh // 128, 384], F32, tag="pg")
            nc.sync.dma_start(tg[:], t_good)
            nc.scalar.dma_start(pg[:], p_good)

            # i side row 4 = -vsq/4 where v = 2t - 1
            v2 = cpool.tile([n // 128, 384], F32, tag="v2")
            nc.vector.tensor_scalar(v2[:], tg[:], SCALE, -SCALE * OFFSET,
                                    op0=ALU.mult, op1=ALU.add)
            vsq = cpool.tile([n // 128, 384], F32, tag="vsq")
            nc.vector.tensor_tensor(vsq[:], v2[:], v2[:], op=ALU.mult)
            va = cpool.tile([n // 128, 128], F32, tag="va")
            nc.vector.tensor_tensor(va[:], vsq[:, 0:128], vsq[:, 128:256],
                                    op=ALU.add)
            vb = cpool.tile([n // 128, 128], F32, tag="vb")
            nc.vector.tensor_tensor(vb[:], va[:], vsq[:, 256:384], op=ALU.add)
            nvs = cpool.tile([n // 128, 128], MM_DT, tag="nvs")
            nc.vector.tensor_scalar_mul(nvs[:], vb[:], -0.25)
            nc.sync.dma_start(L[4:5, :], nvs[:])

            # j side row 3 = -(psq + 2sp)/4 - BIG*invalid
            psq_e = cpool.tile([mh // 128, 384], F32, tag="psq_e")
            nc.vector.tensor_tensor(psq_e[:], pg[:], pg[:], op=ALU.mult)
            pa = cpool.tile([mh // 128, 128], F32, tag="pa")
            nc.vector.tensor_tensor(pa[:], psq_e[:, 0:128], psq_e[:, 128:256],
                                    op=ALU.add)
            psq = cpool.tile([mh // 128, 128], F32, tag="psq")
            nc.vector.tensor_tensor(psq[:], pa[:], psq_e[:, 256:384],
                                    op=ALU.add)
            sa = cpool.tile([mh // 128, 128], F32, tag="sa")
            nc.vector.tensor_tensor(sa[:], pg[:, 0:128], pg[:, 128:256],
                                    op=ALU.add)
            sp = cpool.tile([mh // 128, 128], F32, tag="sp")
            nc.vector.tensor_tensor(sp[:], sa[:], pg[:, 256:384], op=ALU.add)
            psq4 = cpool.tile([mh // 128, 128], F32, tag="psq4")
            nc.vector.tensor_scalar_mul(psq4[:], psq[:], 0.25)
            # nb = (sp * -0.5) - psq/4  =  -(psq + 2*sp)/4
            nb = cpool.tile([mh // 128, 128], F32, tag="nb")
            nc.vector.scalar_tensor_tensor(nb[:], sp[:], -0.5, psq4[:],
                                           op0=ALU.mult, op1=ALU.subtract)
            # invalid point <=> psq == 0 ; r3 = (is0 * -BIG) + nb
            is0 = cpool.tile([mh // 128, 128], F32, tag="is0")
            nc.vector.tensor_scalar(is0[:], psq[:], 0.0, None,
                                    op0=ALU.is_equal)
            r3 = cpool.tile([mh // 128, 128], MM_DT, tag="r3")
            nc.vector.scalar_tensor_tensor(r3[:], is0[:], -BIG, nb[:],
                                           op0=ALU.mult, op1=ALU.add)
            nc.sync.dma_start(R[3:4, :], r3[:])

            # ---- init col-max accumulator ----
            nc.gpsimd.memset(cmax[:], NEG_INIT)

            # absorb the cmax-memset (Pool) semaphore into the DVE clock
            # once, so col-chain TTs carry only their other wait
            pscr = cpool.tile([1, 8], F16, tag="pscr")
            gscr = cpool.tile([1, 8], F32, tag="gscr")
            nc.vector.tensor_copy(pscr[0:1, 0:1], cmax[0:1, 0:1])

            # ---- wait-spreaders: absorb one DMA-queue semaphore each so
            # real matmuls carry <=1 embedded wait (S3_LW struct limit) ----
            wp = pspool.tile([128, G], F32, tag="pt")
            for ap_ in (L[0:4, 0:1], L[0:5, 0:1], R[0:3, 0:1],
                        R[0:4, 0:1], R[0:5, 0:1]):
                nc.tensor.matmul(wp[0:1, 0:1], ap_.bitcast(F32),
                                 ap_.bitcast(F32), start=True, stop=True)
            nc.tensor.matmul(wp[0:1, 0:1], idt[0:1, 0:1], idt[0:1, 0:1],
                             start=True, stop=True)

            # ---- main loop ----
            for it in range(nt):
                sts = []
                for g in range(ng):
                    pt = pspool.tile([128, G], F32, tag="pt")
                    for c in range(gc):
                        j0 = g * G + c * 512
                        nc.tensor.matmul(
                            pt[:, c * 512:(c + 1) * 512],
                            L[0:5, it * 128:(it + 1) * 128],
                            R[0:5, j0:j0 + 512],
                            start=True, stop=True)
                    # ACT: stage psum -> sbuf fp16.  The pre-touch write
                    # (to the disjoint pad column) absorbs the stage-slot
                    # WAR (DVE readers of the tile a few groups ago) so the
                    # real stage carries only the PE wait -- walrus embeds
                    # at most one sem wait per instruction.
                    st = spool.tile([128, G + 8], F16, tag="st")
                    nc.scalar.activation(st[0:1, G:G + 1], idt[0:1, 0:1],
                                         AF.Copy)
                    nc.scalar.activation(st[:, 0:G], pt[:], AF.Copy)
                    # col chain: cmax = max(cmax, staged)  (before the
                    # in-place row folding below mangles st)
                    nc.vector.tensor_tensor(cmax[:, g * G:(g + 1) * G],
                                            cmax[:, g * G:(g + 1) * G],
                                            st[:, 0:G], op=ALU.max)
                    sts.append(st)
                # DVE row chain: pairwise-max folds at the 16-bit 2x TT
                # rate (first fold across the ng staged tiles), then one
                # narrow 1x reduce.  (Full-width reduce is 1x-capped;
                # fused tensor_scalar+accum too; TTR doesn't compile.)
                st = sts[0]
                for o in sts[1:]:
                    nc.vector.tensor_tensor(st[:, 0:G], st[:, 0:G],
                                            o[:, 0:G], op=ALU.max)
                h1, h2, h3 = G // 2, G // 4, G // 8
                nc.vector.tensor_tensor(st[:, 0:h1], st[:, 0:h1],
                                        st[:, h1:G], op=ALU.max)
                nc.vector.tensor_tensor(st[:, 0:h2], st[:, 0:h2],
                                        st[:, h2:h1], op=ALU.max)
                nc.vector.tensor_tensor(st[:, 0:h3], st[:, 0:h3],
                                        st[:, h3:h2], op=ALU.max)
                nc.vector.tensor_reduce(
                    d1buf[:, it:it + 1], st[:, 0:h3],
                    axis=mybir.AxisListType.X, op=ALU.max)

            # ---- tail: per-j max of cmax via PE transpose + DVE reduce.
            # 4 transposes share one PSUM tile; one 3D-AP reduce covers all
            # 4 (amortizes the 120-cycle PSUM-read init per DVE op).
            tb = 4
            for t0 in range(0, jt, tb):
                k = min(tb, jt - t0)
                ptt = pspool.tile([128, 128 * tb], F16, tag="pt")
                for q in range(k):
                    nc.tensor.transpose(
                        ptt[:, q * 128:(q + 1) * 128],
                        cmax[:, (t0 + q) * 128:(t0 + q + 1) * 128], idt[:])
                nc.vector.tensor_reduce(
                    d2buf[:, t0:t0 + k],
                    ptt[:, 0:k * 128].rearrange("p (a b) -> p a b", b=128),
                    axis=mybir.AxisListType.X, op=ALU.max)

            nc.gpsimd.dma_start(outneg, obuf[:])

    strip_redundant_waits(nc)
    return nc


def strip_redundant_waits(nc):
    """Transitively-implied semaphore-wait elimination.

    Tile emits per-instruction wait lists without transitive reduction
    (documented: "Tile doesn't track that syncing on engine X told us
    about Y").  walrus's fp32-matmul lowering (S3_LW) and direct2d DMA
    structs can embed only ONE wait, so a slot-reuse matmul carrying
    [ACT>=a, PE>=p] fails codegen even though the PE wait is implied by
    the ACT wait (the ACT instruction itself waited on PE>=p).

    Soundness: a wait (S>=v) may be dropped iff it is guaranteed by the
    union of (a) knowledge inherited from the previous instruction on
    the same in-order engine, and (b) completion-knowledge of the
    instructions that perform the other waits' target increments.
    Completion of an in-order engine's instruction implies completion
    (and sem updates) of all earlier instructions on that engine.  DMA
    transfers complete out of order w.r.t. the issuing engine, so each
    DMA instruction is its own "engine".
    """
    import concourse.mybir as mb

    insts = []
    for blk in nc.m.functions[0].blocks:
        insts.extend(list(blk.instructions))
    if True:
        n = len(insts)
        # engine key per instruction (DMA transfers are their own proc)
        ekeys = []
        for idx, i in enumerate(insts):
            if type(i).__name__ in ("InstDMACopy", "InstLoad", "InstSave"):
                ekeys.append(("dma", idx))
            else:
                ekeys.append(("eng", str(getattr(i, "engine", idx))))
        prev_on_eng = {}
        prev_idx = [None] * n
        for idx in range(n):
            k = ekeys[idx]
            prev_idx[idx] = prev_on_eng.get(k)
            prev_on_eng[k] = idx
        # cumulative sem updates in schedule order; sems that are ever
        # decremented or register-updated are excluded (non-monotone).
        bad_sems = set()
        for i in insts:
            si = i.sync_info
            if not si:
                continue
            for u in si.on_update:
                if u.update_mode not in ("sem-add-imm", "sem-inc")                         or u.update_reg is not None:
                    bad_sems.add(u.ant_name)
        upd_timeline = {}
        cums = {}
        upd_of = [None] * n  # idx -> list[(sem, cum_after)]
        for idx, i in enumerate(insts):
            si = i.sync_info
            if not si:
                upd_of[idx] = []
                continue
            ups = []
            for u in si.on_update:
                if u.ant_name in bad_sems:
                    continue
                amt = 1 if u.update_mode == "sem-inc" else u.update_value
                c = cums.get(u.ant_name, 0) + amt
                cums[u.ant_name] = c
                upd_timeline.setdefault(u.ant_name, []).append((c, idx))
                ups.append((u.ant_name, c))
            upd_of[idx] = ups

        def inc_idx(sem, v):
            tl = upd_timeline.get(sem)
            if not tl:
                return None
            for c, idx in tl:
                if c >= v:
                    return idx
            return None

        D_cache = {}
        C_cache = {}

        def merge(dst, src):
            for s, v in src.items():
                if dst.get(s, -1) < v:
                    dst[s] = v

        def D(idx):
            if idx in D_cache:
                return D_cache[idx]
            D_cache[idx] = {}   # cycle guard
            out = {}
            p = prev_idx[idx]
            if p is not None:
                merge(out, D(p))
            si = insts[idx].sync_info
            if si:
                for w in si.on_wait:
                    if w.wait_mode != "sem-ge-imm" or w.wait_reg is not None                             or w.ant_name in bad_sems:
                        continue
                    j = inc_idx(w.ant_name, w.wait_value)
                    if j is not None and j < idx:
                        merge(out, C(j))
                    if out.get(w.ant_name, -1) < w.wait_value:
                        out[w.ant_name] = w.wait_value
            D_cache[idx] = out
            return out

        def C(idx):
            if idx in C_cache:
                return C_cache[idx]
            C_cache[idx] = {}   # cycle guard
            out = dict(D(idx))
            # completion of idx implies completion of all earlier same-eng
            k = ekeys[idx]
            j = idx
            while j is not None:
                for s, c in upd_of[j]:
                    if out.get(s, -1) < c:
                        out[s] = c
                j = prev_idx[j]
            C_cache[idx] = out
            return out

        for idx, i in enumerate(insts):
            si = i.sync_info
            if not si or len(si.on_wait) <= 1:
                continue
            waits = list(si.on_wait)
            if any(w.wait_mode != "sem-ge-imm" or w.wait_reg is not None
                   for w in waits):
                continue
            keep = []
            for wi, w in enumerate(waits):
                if w.ant_name in bad_sems:
                    keep.append(w)
                    continue
                know = {}
                p = prev_idx[idx]
                if p is not None:
                    merge(know, D(p))
                for wj, w2 in enumerate(waits):
                    if wj == wi or w2.ant_name in bad_sems:
                        continue
                    j = inc_idx(w2.ant_name, w2.wait_value)
                    if j is not None and j < idx:
                        merge(know, C(j))
                    if know.get(w2.ant_name, -1) < w2.wait_value:
                        know[w2.ant_name] = w2.wait_value
                if know.get(w.ant_name, -1) >= w.wait_value:
                    continue    # implied -> drop
                keep.append(w)
            if len(keep) < len(waits):
                i.sync_info = mb.SyncInfo(on_wait=keep,
                                          on_update=list(si.on_update))


_NC_CACHE = {}


def _get_nc(n=N, mh=MH):
    key = (n, mh)
    if key not in _NC_CACHE:
        _NC_CACHE[key] = build_nc(n, mh)
    return _NC_CACHE[key]


def make_in_maps(vertices, pc, n=N, mh=MH):
    vertices = np.asarray(vertices)
    pc = np.asarray(pc)
    b_total = vertices.shape[0]
    top = vertices[:, :, :, -1, :].reshape(b_total, 3, -1)[:, :, :n]
    top = np.ascontiguousarray(top, dtype=np.float32)
    ident = np.eye(128, dtype=np.float16)
    in_maps = []
    for c in range(N_CORES):
        b, h = divmod(c, 2)
        b = b % b_total
        t_raw = top[b]
        p_raw = np.ascontiguousarray(pc[b][:, h * mh:(h + 1) * mh],
                                     dtype=np.float32)
        l_base = np.zeros((5, n), np.float32)
        l_base[0:3] = t_raw
        l_base[3] = 1.0
        r_base = np.zeros((5, mh), np.float32)
        r_base[0:3] = p_raw
        r_base[4] = 1.0
        t_g = np.ascontiguousarray(
            t_raw.reshape(3, n // 128, 128).transpose(1, 0, 2)
            .reshape(n // 128, 384))
        p_g = np.ascontiguousarray(
            p_raw.reshape(3, mh // 128, 128).transpose(1, 0, 2)
            .reshape(mh // 128, 384))
        in_maps.append({
            "l_base": l_base, "r_base": r_base,
            "t_good": t_g, "p_good": p_g,
            "ident": ident,
        })
    return in_maps


def combine(results, pc, n=N, mh=MH):
    """Combine per-core [128, nt]/[128, jt] outputs (of -d/4) into the loss."""
    pc = np.asarray(pc)
    losses = []
    for b in range(pc.shape[0]):
        nt, jt = n // 128, mh // 128
        r0, r1 = results[2 * b], results[2 * b + 1]
        rneg = np.maximum(r0["outneg"][:, 0:nt], r1["outneg"][:, 0:nt])
        dist1 = (-4.0 * rneg.T.reshape(n)).astype(np.float64)
        dist2 = np.concatenate([
            (-4.0 * r["outneg"][:, nt:nt + jt].T.reshape(mh))
            .astype(np.float64) for r in (r0, r1)])
        mask = ~np.all(pc[b] == 0.0, axis=0)
        n_valid = max(int(mask.sum()), 1)
        losses.append(dist1.mean() + dist2[mask].sum() / n_valid)
    return np.asarray(np.mean(losses), dtype=np.float32)


def kernel(vertices, pc):
    nc = _get_nc()
    in_maps = make_in_maps(vertices, pc)
    res = run_bass_kernel_spmd(nc, in_maps, list(range(N_CORES))).results
    return combine(res, pc)
